# revision 5
# baseline (speedup 1.0000x reference)
"""DifferentialAttention Trainium2 kernel (v4 — flattened per-head pipeline).

Sharding: 8 cores = 2 (batch) x 4 (head groups of 4 heads).
Each core computes, for its (b, head-group): QKV projection ->
differential attention (2 softmaxes per head) -> partial output
projection (its 512 rows of w_proj). Host sums the 4 partials per
batch element and adds b_proj.

Schedule (v4): the exp stream on ACT (~260us/rep) is the second-longest
engine load after PE (~390us), so the kernel runs a single flat pipeline
that keeps ACT fed from ~80us into the rep until the end:

  V sweep -> K/Q(h0) -> for h in 0..3: stage(h)
  stage(h) = scores(blk,qc)+exp for head h interleaved with AV of the
  previous (blk,qc) chunk, with the K/Q GEMMs of head h+1 (or, for the
  last stage, the blk0 projection) woven between score tiles as PE
  filler.  Projection of blk1 forms the rep tail; the next rep's V
  sweep x-chunks/weights prefetch under it.

  - per-head K/Q tiles ([128,2048] bf16, 2-deep) instead of a monolithic
    [dh,h,s] store: kills the cross-rep write-after-read serialization.
  - PSUM partitioned: 2 banks GEMM accumulators, 4 banks score tiles
    ([128,1024]x2), 2 banks AV/proj/transpose - no pool aliasing, so
    consecutive reps' GEMMs never wait on last-rep attention psum.
  - scores packed: att0/att1 matmuls contract over disjoint 64-partition
    ranges (PE row groups 0/64) and are issued adjacently.
  - dtypes as v1: bf16 GEMMs/Q/K/es-stationary AV, V fp16 with ones
    column (softmax denominator), exp fp16 with -10 bias, fp32 psum and
    normalize, fp32 output partials.
  - output stores go out on the ACT HWDGE ring (nc.scalar.dma_start);
    all loads on the SP ring - the 16MB/rep store stream never queues
    behind next rep's x prefetch.
"""

import os

_jp = os.environ.get("JAX_PLATFORMS")
if _jp is not None and "axon" not in _jp:
    os.environ["JAX_PLATFORMS"] = "axon," + _jp

import contextlib

import ml_dtypes
import numpy as np

import concourse.bass as bass
import concourse.tile as tile
from concourse import bacc, mybir
from concourse.bass_utils import run_bass_kernel_spmd
from concourse.masks import make_identity

DIM = 2048
S = 2048
NHEAD_G = 4            # heads per core
DH = 128
HALF = 64
SCALE = DH ** -0.5

F32 = mybir.dt.float32
F16 = mybir.dt.float16
BF16 = mybir.dt.bfloat16

KT = DIM // 128        # 16 contraction tiles
SKT = S // 128         # 16 key tiles
NBLK = 2               # s_q blocks of 1024
BLK = S // NBLK        # 1024
SQT = BLK // 128       # 8 s_q tiles per block


def _pull(gen, n=1):
    for _ in range(n):
        try:
            next(gen)
        except StopIteration:
            return False
    return True


def build_program(reps=1):
    nc = bacc.Bacc(None, target_bir_lowering=False, debug=False)

    xTb = nc.dram_tensor("xTb", [DIM, S], BF16, kind="ExternalInput").ap()
    wq = nc.dram_tensor("wq", [DIM, NHEAD_G * DH], BF16, kind="ExternalInput").ap()
    wk = nc.dram_tensor("wk", [DIM, NHEAD_G * DH], BF16, kind="ExternalInput").ap()
    wvb = nc.dram_tensor("wvb", [DIM, NHEAD_G * DH], BF16, kind="ExternalInput").ap()
    wpb = nc.dram_tensor("wpb", [NHEAD_G * DH, DIM], BF16, kind="ExternalInput").ap()
    neg_lam = nc.dram_tensor("neg_lam", [1, 1], F32, kind="ExternalInput").ap()
    out = nc.dram_tensor("out", [S, DIM], F32, kind="ExternalOutput").ap()

    xTb_t = xTb.rearrange("(kt p) s -> p kt s", p=128)        # [128, KT, S]
    wq_t = wq.rearrange("(kt p) c -> p kt c", p=128)          # [128, KT, 512]
    wk_t = wk.rearrange("(kt p) c -> p kt c", p=128)
    wvb_t = wvb.rearrange("(kt p) c -> p kt c", p=128)
    wpb_t = wpb.rearrange("(kt p) c -> p kt c", p=128)        # [128, 4, DIM]

    with tile.TileContext(nc) as tc:
        with (
            tc.tile_pool(name="persist", bufs=1) as persist,
            tc.tile_pool(name="kq", bufs=2) as kqp,
            tc.tile_pool(name="es", bufs=30) as esp,
            tc.tile_pool(name="xs", bufs=2) as xsp,
            tc.tile_pool(name="wkq", bufs=3) as wkqp,
            tc.tile_pool(name="ot", bufs=1) as otp,
            tc.tile_pool(name="u", bufs=2) as up,
            tc.tile_pool(name="ob", bufs=4) as obp,
            tc.tile_pool(name="outs", bufs=6) as outsp,
            tc.tile_pool(name="rp", bufs=8) as rp,
            tc.tile_pool(name="gp", bufs=2, space="PSUM") as gp,
            tc.tile_pool(name="psA", bufs=2, space="PSUM") as psA,
            tc.tile_pool(name="psB", bufs=2, space="PSUM") as psB,
        ):
            V = persist.tile([128, SKT, NHEAD_G, DH + 1], F16, tag="V")
            ident = persist.tile([128, 128], BF16, tag="ident")
            nlam = persist.tile([128, 1], F32, tag="nlam")
            bias10 = persist.tile([128, 1], F32, tag="bias10")
            nc.gpsimd.memset(bias10[:], -10.0)
            nc.gpsimd.memset(V[:, :, :, DH:DH + 1], 1.0)
            make_identity(nc, ident[:])
            nc.sync.dma_start(out=nlam[:], in_=neg_lam.to_broadcast([128, 1]))

            def make_head_tiles(h):
                KTh = kqp.tile([128, S], BF16, tag="K", name=f"K{h}")
                QTh = kqp.tile([128, S], BF16, tag="Q", name=f"Q{h}")
                wkh = wkqp.tile([128, KT, DH], BF16, tag="w", name=f"wk{h}")
                wqh = wkqp.tile([128, KT, DH], BF16, tag="w", name=f"wq{h}")
                return KTh, QTh, wkh, wqh

            def gemm_head_gen(h, tiles):
                # sc chunks paired so each stationary (wt[:, k]) feeds two
                # matmuls back-to-back - the second skips the weight load
                KTh, QTh, wkh, wqh = tiles
                nc.sync.dma_start(out=wkh[:], in_=wk_t[:, :, h * DH:(h + 1) * DH])
                nc.sync.dma_start(out=wqh[:], in_=wq_t[:, :, h * DH:(h + 1) * DH])
                for scp in range(2):
                    xcs = []
                    for sc in (2 * scp, 2 * scp + 1):
                        xc = xsp.tile([128, KT, 512], BF16, tag="x",
                                      name=f"x{h}_{sc}")
                        for hf in range(2):
                            kc = slice(hf * 8, (hf + 1) * 8)
                            nc.sync.dma_start(
                                out=xc[:, kc],
                                in_=xTb_t[:, kc, sc * 512:(sc + 1) * 512])
                        xcs.append(xc)
                    for wt, dst in ((wkh, KTh), (wqh, QTh)):
                        pss = [gp.tile([128, 512], F32, tag="g",
                                       name=f"g{h}{scp}{i}") for i in range(2)]
                        for k in range(KT):
                            for i in range(2):
                                nc.tensor.matmul(pss[i][:], wt[:, k], xcs[i][:, k],
                                                 start=(k == 0), stop=(k == KT - 1))
                            if k % 2 == 1:
                                yield
                        for i in range(2):
                            sc = 2 * scp + i
                            nc.vector.tensor_copy(
                                dst[:, sc * 512:(sc + 1) * 512], pss[i][:])
                            yield

            def score_tile(KTh, QTh, blk, qc, kt):
                sps = psA.tile([128, 1024], F32, tag="sA")
                qsl = slice(blk * BLK + qc * 512, blk * BLK + (qc + 1) * 512)
                for att in range(2):
                    dsl = slice(att * HALF, (att + 1) * HALF)
                    nc.tensor.matmul(sps[:, att * 512:(att + 1) * 512],
                                     KTh[dsl, kt * 128:(kt + 1) * 128],
                                     QTh[dsl, qsl], start=True, stop=True)
                es = esp.tile([128, 1024], F16, tag="es")
                nc.scalar.activation(es[:], sps[:],
                                     mybir.ActivationFunctionType.Exp,
                                     bias=bias10[:])
                return es

            def av_group(es_list, h, att, sq, usb):
                ups = psB.tile([128, 512], F32, tag="pB")
                ssl = slice(att * 512 + (sq % 4) * 128,
                            att * 512 + (sq % 4 + 1) * 128)
                for kt in range(SKT):
                    nc.tensor.matmul(ups[:, 0:DH + 1],
                                     es_list[kt][:, ssl],
                                     V[:, kt, h, :],
                                     start=(kt == 0), stop=(kt == SKT - 1))
                nc.vector.tensor_copy(usb[:, sq, 0:DH + 1], ups[:, 0:DH + 1])

            loop_cm = tc.For_i(0, reps, 1) if reps > 1 else contextlib.nullcontext()
            with loop_cm:
                # ---------------- V sweep ----------------
                with tc.tile_pool(name="wv", bufs=1) as wvp:
                    wv = wvp.tile([128, KT, 512], BF16, tag="wv")
                    for kc4 in range(4):
                        kc = slice(kc4 * 4, (kc4 + 1) * 4)
                        nc.sync.dma_start(out=wv[:, kc], in_=wvb_t[:, kc])
                    for g in range(4):
                        xc = xsp.tile([128, KT, 512], BF16, tag="x", name=f"xv{g}")
                        for hf in range(2):
                            kc = slice(hf * 8, (hf + 1) * 8)
                            nc.sync.dma_start(
                                out=xc[:, kc],
                                in_=xTb_t[:, kc, g * 512:(g + 1) * 512])
                        for mt in range(4):
                            vp = gp.tile([128, 512], F32, tag="g")
                            for k in range(KT):
                                nc.tensor.matmul(
                                    vp[:],
                                    xc[:, k, mt * 128:(mt + 1) * 128],
                                    wv[:, k],
                                    start=(k == 0), stop=(k == KT - 1))
                            nc.vector.tensor_copy(
                                V[:, g * 4 + mt, :, 0:DH],
                                vp.rearrange("p (h d) -> p h d", h=NHEAD_G))

                with tc.tile_pool(name="wp", bufs=1) as wpp:
                    # K/Q for head 0 (prologue, unfilled)
                    cur = make_head_tiles(0)
                    for _ in gemm_head_gen(0, cur):
                        pass

                    OT0 = otp.tile([128, NHEAD_G, BLK], BF16, tag="OT0")
                    OT1 = otp.tile([128, NHEAD_G, BLK], BF16, tag="OT1")
                    wp = None

                    def proj_gen(OT, blk):
                        for nb in range(4):
                            nsl = slice(nb * 512, (nb + 1) * 512)
                            for mt in range(SQT):
                                msl = slice(blk * BLK + mt * 128,
                                            blk * BLK + (mt + 1) * 128)
                                pps = psB.tile([128, 512], F32, tag="pB")
                                for k in range(NHEAD_G):
                                    nc.tensor.matmul(
                                        pps[:],
                                        OT[:, k, mt * 128:(mt + 1) * 128],
                                        wp[:, k, nsl],
                                        start=(k == 0), stop=(k == NHEAD_G - 1))
                                ot = outsp.tile([128, 512], F32, tag="os")
                                nc.vector.tensor_copy(ot[:], pps[:])
                                nc.scalar.dma_start(out=out[msl, nsl], in_=ot[:])
                                yield

                    def stage(h, KTh, QTh, fill_early, fill_late):
                        es_l = {}
                        u = {}

                        def get_u(blk):
                            if blk not in u:
                                u1sb = up.tile([128, SQT, DH + 4], F32, tag="u1",
                                               name=f"u1_{h}_{blk}")
                                u2sb = up.tile([128, SQT, DH + 4], F32, tag="u2",
                                               name=f"u2_{h}_{blk}")
                                u[blk] = (u1sb, u2sb)
                            return u[blk]

                        def norm(blk):
                            u1sb, u2sb = u[blk]
                            r1 = rp.tile([128, SQT], F32, tag="r")
                            nc.vector.reciprocal(r1[:], u1sb[:, :, DH])
                            r2n = rp.tile([128, SQT], F32, tag="r")
                            nc.vector.reciprocal(r2n[:], u2sb[:, :, DH])
                            nc.vector.tensor_scalar_mul(r2n[:], r2n[:], nlam[:])
                            OT = OT0 if blk == 0 else OT1
                            for sq in range(SQT):
                                o1 = obp.tile([128, DH], F32, tag="o")
                                nc.vector.tensor_scalar_mul(
                                    o1[:], u1sb[:, sq, 0:DH], r1[:, sq:sq + 1])
                                o2 = obp.tile([128, DH], F32, tag="o")
                                nc.vector.tensor_scalar_mul(
                                    o2[:], u2sb[:, sq, 0:DH], r2n[:, sq:sq + 1])
                                oc = obp.tile([128, DH], BF16, tag="oc")
                                nc.vector.tensor_add(oc[:], o1[:], o2[:])
                                tps = psB.tile([128, 256], BF16, tag="pB")
                                nc.tensor.transpose(tps[:, 0:128], oc[:], ident[:])
                                nc.vector.tensor_copy(
                                    OT[:, h, sq * 128:(sq + 1) * 128],
                                    tps[:, 0:128])

                        def sec(s_key, a_key, f, tail_pulls=1):
                            new = [] if s_key else None
                            if a_key:
                                a_blk, a_qc = a_key
                                u1sb, u2sb = get_u(a_blk)
                                av_list = es_l[a_key]
                            for i in range(8):
                                if s_key:
                                    s_blk, s_qc = s_key
                                    new.append(score_tile(KTh, QTh, s_blk, s_qc,
                                                          2 * i))
                                    new.append(score_tile(KTh, QTh, s_blk, s_qc,
                                                          2 * i + 1))
                                if a_key:
                                    att, sql = divmod(i, 4)
                                    sq = a_qc * 4 + sql
                                    usb = u1sb if att == 0 else u2sb
                                    av_group(av_list, h, att, sq, usb)
                                _pull(f, tail_pulls)
                            if s_key:
                                es_l[s_key] = new

                        # S(0,0) with filler
                        es00 = []
                        for kt in range(SKT):
                            es00.append(score_tile(KTh, QTh, 0, 0, kt))
                            _pull(fill_early, 2)
                        es_l[(0, 0)] = es00
                        sec((0, 1), (0, 0), fill_early)
                        sec((1, 0), (0, 1), fill_early)
                        norm(0)
                        sec((1, 1), (1, 0), fill_late)
                        sec(None, (1, 1), fill_late, tail_pulls=3)
                        norm(1)

                    for h in range(NHEAD_G):
                        if h < NHEAD_G - 1:
                            nxt = make_head_tiles(h + 1)
                            fe = gemm_head_gen(h + 1, nxt)
                            fl = fe
                        else:
                            wp = wpp.tile([128, NHEAD_G, DIM], BF16, tag="wp")
                            for hh in range(NHEAD_G):
                                nc.sync.dma_start(out=wp[:, hh], in_=wpb_t[:, hh])
                            fe = iter(())
                            fl = proj_gen(OT0, 0)
                        stage(h, cur[0], cur[1], fe, fl)
                        while _pull(fe):
                            pass
                        if h == NHEAD_G - 1:
                            while _pull(fl):
                                pass
                        else:
                            cur = nxt
                    # blk1 projection tail: nb pairs share each OT stationary
                    for nbp in range(2):
                        for mt in range(SQT):
                            msl = slice(BLK + mt * 128, BLK + (mt + 1) * 128)
                            pps = [psB.tile([128, 512], F32, tag="pB",
                                            name=f"pt{nbp}{mt}{i}")
                                   for i in range(2)]
                            for k in range(NHEAD_G):
                                for i in range(2):
                                    nsl = slice((2 * nbp + i) * 512,
                                                (2 * nbp + i + 1) * 512)
                                    nc.tensor.matmul(
                                        pps[i][:],
                                        OT1[:, k, mt * 128:(mt + 1) * 128],
                                        wp[:, k, nsl],
                                        start=(k == 0), stop=(k == NHEAD_G - 1))
                            for i in range(2):
                                nsl = slice((2 * nbp + i) * 512,
                                            (2 * nbp + i + 1) * 512)
                                ot = outsp.tile([128, 512], F32, tag="os")
                                nc.vector.tensor_copy(ot[:], pps[i][:])
                                nc.scalar.dma_start(out=out[msl, nsl], in_=ot[:])

    nc.compile()
    return nc


_CACHE = {}


def _get_program(reps=1):
    key = f"nc{reps}"
    if key not in _CACHE:
        _CACHE[key] = build_program(reps)
    return _CACHE[key]


def shard_inputs(inputs):
    """Full-input dict -> per-core in_maps for run_bass_kernel_spmd."""
    x = np.asarray(inputs["x"], dtype=np.float32)
    w_qkv = np.asarray(inputs["w_qkv"], dtype=np.float32)
    w_proj = np.asarray(inputs["w_proj"], dtype=np.float32)
    lambda_q1 = np.asarray(inputs["lambda_q1"], dtype=np.float32)
    lambda_k1 = np.asarray(inputs["lambda_k1"], dtype=np.float32)
    lambda_q2 = np.asarray(inputs["lambda_q2"], dtype=np.float32)
    lambda_k2 = np.asarray(inputs["lambda_k2"], dtype=np.float32)
    li = np.float32(np.asarray(inputs["layer_idx"]))

    B = x.shape[0]
    H = 16

    layer_factor = np.clip(li * np.float32(0.3), np.float32(0.0), np.float32(5.0))
    lam_init = np.float32(0.8) - np.float32(0.6) * np.exp(-layer_factor)
    l1 = np.clip(np.sum(lambda_q1 * lambda_k1), -10.0, 10.0).astype(np.float32)
    l2 = np.clip(np.sum(lambda_q2 * lambda_k2), -10.0, 10.0).astype(np.float32)
    lam = np.clip(np.exp(l1) - np.exp(l2) + lam_init, 0.1, 5.0).astype(np.float32)

    xT = [np.ascontiguousarray(x[b].T) for b in range(B)]
    xTb = [t.astype(ml_dtypes.bfloat16) for t in xT]
    neg_lam = np.array([[-lam]], dtype=np.float32)

    in_maps = []
    for c in range(8):
        b = c // 4
        g = c % 4
        h0 = g * NHEAD_G
        cq = slice(h0 * DH, (h0 + NHEAD_G) * DH)
        ck = slice(H * DH + h0 * DH, H * DH + (h0 + NHEAD_G) * DH)
        cv = slice(2 * H * DH + h0 * DH, 2 * H * DH + (h0 + NHEAD_G) * DH)
        in_maps.append({
            "xTb": xTb[b],
            "wq": (np.ascontiguousarray(w_qkv[:, cq])
                   * np.float32(SCALE)).astype(ml_dtypes.bfloat16),
            "wk": np.ascontiguousarray(w_qkv[:, ck]).astype(ml_dtypes.bfloat16),
            "wvb": np.ascontiguousarray(w_qkv[:, cv]).astype(ml_dtypes.bfloat16),
            "wpb": np.ascontiguousarray(
                w_proj[h0 * DH:(h0 + NHEAD_G) * DH, :]).astype(ml_dtypes.bfloat16),
            "neg_lam": neg_lam,
        })
    return in_maps


def kernel(x, w_qkv, w_proj, b_proj, lambda_q1, lambda_k1, lambda_q2, lambda_k2,
           layer_idx):
    inputs = dict(x=x, w_qkv=w_qkv, w_proj=w_proj, b_proj=b_proj,
                  lambda_q1=lambda_q1, lambda_k1=lambda_k1,
                  lambda_q2=lambda_q2, lambda_k2=lambda_k2, layer_idx=layer_idx)
    in_maps = shard_inputs(inputs)
    b_proj = np.asarray(b_proj, dtype=np.float32)
    B = np.asarray(x).shape[0]

    nc = _get_program()
    last_err = None
    for attempt in range(3):
        try:
            res = run_bass_kernel_spmd(nc, in_maps, list(range(8)))
            break
        except Exception as e:  # noqa: BLE001
            last_err = e
    else:
        raise last_err

    out = np.empty((B, S, DIM), dtype=np.float32)
    for b in range(B):
        acc = res.results[4 * b]["out"].copy()
        for g in range(1, 4):
            acc += res.results[4 * b + g]["out"]
        out[b] = acc + b_proj
    return out


# revision 11
# speedup vs baseline: 1.0120x; 1.0120x over previous
"""DifferentialAttention Trainium2 kernel (v4 — flattened per-head pipeline).

Sharding: 8 cores = 2 (batch) x 4 (head groups of 4 heads).
Each core computes, for its (b, head-group): QKV projection ->
differential attention (2 softmaxes per head) -> partial output
projection (its 512 rows of w_proj). Host sums the 4 partials per
batch element and adds b_proj.

Schedule (v4): the exp stream on ACT (~260us/rep) is the second-longest
engine load after PE (~390us), so the kernel runs a single flat pipeline
that keeps ACT fed from ~80us into the rep until the end:

  V sweep -> K/Q(h0) -> for h in 0..3: stage(h)
  stage(h) = scores(blk,qc)+exp for head h interleaved with AV of the
  previous (blk,qc) chunk, with the K/Q GEMMs of head h+1 (or, for the
  last stage, the blk0 projection) woven between score tiles as PE
  filler.  Projection of blk1 forms the rep tail; the next rep's V
  sweep x-chunks/weights prefetch under it.

  - per-head K/Q tiles ([128,2048] bf16, 2-deep) instead of a monolithic
    [dh,h,s] store: kills the cross-rep write-after-read serialization.
  - PSUM partitioned: 2 banks GEMM accumulators, 4 banks score tiles
    ([128,1024]x2), 2 banks AV/proj/transpose - no pool aliasing, so
    consecutive reps' GEMMs never wait on last-rep attention psum.
  - scores packed: att0/att1 matmuls contract over disjoint 64-partition
    ranges (PE row groups 0/64) and are issued adjacently.
  - dtypes as v1: bf16 GEMMs/Q/K/es-stationary AV, V fp16 with ones
    column (softmax denominator), exp fp16 with -10 bias, fp32 psum and
    normalize, fp32 output partials.
  - output stores go out on the ACT HWDGE ring (nc.scalar.dma_start);
    all loads on the SP ring - the 16MB/rep store stream never queues
    behind next rep's x prefetch.
"""

import os

_jp = os.environ.get("JAX_PLATFORMS")
if _jp is not None and "axon" not in _jp:
    os.environ["JAX_PLATFORMS"] = "axon," + _jp

import contextlib

import ml_dtypes
import numpy as np

import concourse.bass as bass
import concourse.tile as tile
from concourse import bacc, mybir
from concourse.bass_utils import run_bass_kernel_spmd
from concourse.masks import make_identity

DIM = 2048
S = 2048
NHEAD_G = 4            # heads per core
DH = 128
HALF = 64
SCALE = DH ** -0.5

F32 = mybir.dt.float32
F16 = mybir.dt.float16
BF16 = mybir.dt.bfloat16

KT = DIM // 128        # 16 contraction tiles
SKT = S // 128         # 16 key tiles
NBLK = 2               # s_q blocks of 1024
BLK = S // NBLK        # 1024
SQT = BLK // 128       # 8 s_q tiles per block


def _pull(gen, n=1):
    for _ in range(n):
        try:
            next(gen)
        except StopIteration:
            return False
    return True


def build_program(reps=1):
    nc = bacc.Bacc(None, target_bir_lowering=False, debug=False)

    xTb = nc.dram_tensor("xTb", [DIM, S], BF16, kind="ExternalInput").ap()
    wq = nc.dram_tensor("wq", [DIM, NHEAD_G * DH], BF16, kind="ExternalInput").ap()
    wk = nc.dram_tensor("wk", [DIM, NHEAD_G * DH], BF16, kind="ExternalInput").ap()
    wvb = nc.dram_tensor("wvb", [DIM, NHEAD_G * DH], BF16, kind="ExternalInput").ap()
    wpb = nc.dram_tensor("wpb", [NHEAD_G * DH, DIM], BF16, kind="ExternalInput").ap()
    neg_lam = nc.dram_tensor("neg_lam", [1, 1], F32, kind="ExternalInput").ap()
    out = nc.dram_tensor("out", [S, DIM], F32, kind="ExternalOutput").ap()

    xTb_t = xTb.rearrange("(kt p) s -> p kt s", p=128)        # [128, KT, S]
    wq_t = wq.rearrange("(kt p) c -> p kt c", p=128)          # [128, KT, 512]
    wk_t = wk.rearrange("(kt p) c -> p kt c", p=128)
    wvb_t = wvb.rearrange("(kt p) c -> p kt c", p=128)
    wpb_t = wpb.rearrange("(kt p) c -> p kt c", p=128)        # [128, 4, DIM]

    with tile.TileContext(nc) as tc:
        with (
            tc.tile_pool(name="persist", bufs=1) as persist,
            tc.tile_pool(name="kq", bufs=2) as kqp,
            tc.tile_pool(name="es", bufs=30) as esp,
            tc.tile_pool(name="xs", bufs=2) as xsp,
            tc.tile_pool(name="wkq", bufs=3) as wkqp,
            tc.tile_pool(name="ot", bufs=1) as otp,
            tc.tile_pool(name="u", bufs=2) as up,
            tc.tile_pool(name="ob", bufs=4) as obp,
            tc.tile_pool(name="oc", bufs=18) as ocp,
            tc.tile_pool(name="outs", bufs=6) as outsp,
            tc.tile_pool(name="rp", bufs=8) as rp,
            tc.tile_pool(name="gp", bufs=2, space="PSUM") as gp,
            tc.tile_pool(name="psA", bufs=2, space="PSUM") as psA,
            tc.tile_pool(name="psB", bufs=2, space="PSUM") as psB,
        ):
            V = persist.tile([128, SKT, NHEAD_G, DH + 1], F16, tag="V")
            ident = persist.tile([128, 128], BF16, tag="ident")
            nlam = persist.tile([128, 1], F32, tag="nlam")
            bias10 = persist.tile([128, 1], F32, tag="bias10")
            nc.gpsimd.memset(bias10[:], -10.0)
            nc.gpsimd.memset(V[:, :, :, DH:DH + 1], 1.0)
            make_identity(nc, ident[:])
            nc.sync.dma_start(out=nlam[:], in_=neg_lam.to_broadcast([128, 1]))

            def make_head_tiles(h):
                KTh = kqp.tile([128, S], BF16, tag="K", name=f"K{h}")
                QTh = kqp.tile([128, S], BF16, tag="Q", name=f"Q{h}")
                wkh = wkqp.tile([128, KT, DH], BF16, tag="w", name=f"wk{h}")
                wqh = wkqp.tile([128, KT, DH], BF16, tag="w", name=f"wq{h}")
                return KTh, QTh, wkh, wqh

            def gemm_head_gen(h, tiles):
                KTh, QTh, wkh, wqh = tiles
                nc.sync.dma_start(out=wkh[:], in_=wk_t[:, :, h * DH:(h + 1) * DH])
                nc.sync.dma_start(out=wqh[:], in_=wq_t[:, :, h * DH:(h + 1) * DH])
                for sc in range(4):
                    xc = xsp.tile([128, KT, 512], BF16, tag="x", name=f"x{h}_{sc}")
                    for hf in range(2):
                        kc = slice(hf * 8, (hf + 1) * 8)
                        nc.sync.dma_start(
                            out=xc[:, kc],
                            in_=xTb_t[:, kc, sc * 512:(sc + 1) * 512])
                    for wt, dst in ((wkh, KTh), (wqh, QTh)):
                        ps = gp.tile([128, 512], F32, tag="g")
                        for k in range(KT):
                            nc.tensor.matmul(ps[:], wt[:, k], xc[:, k],
                                             start=(k == 0), stop=(k == KT - 1))
                            if k % 4 == 3:
                                yield
                        nc.vector.tensor_copy(dst[:, sc * 512:(sc + 1) * 512], ps[:])
                        yield

            def score_tile(KTh, QTh, blk, qc, kt):
                sps = psA.tile([128, 1024], F32, tag="sA")
                qsl = slice(blk * BLK + qc * 512, blk * BLK + (qc + 1) * 512)
                for att in range(2):
                    dsl = slice(att * HALF, (att + 1) * HALF)
                    nc.tensor.matmul(sps[:, att * 512:(att + 1) * 512],
                                     KTh[dsl, kt * 128:(kt + 1) * 128],
                                     QTh[dsl, qsl], start=True, stop=True)
                es = esp.tile([128, 1024], F16, tag="es")
                nc.scalar.activation(es[:], sps[:],
                                     mybir.ActivationFunctionType.Exp,
                                     bias=bias10[:])
                return es

            def av_group(es_list, h, att, sq, usb):
                ups = psB.tile([128, 512], F32, tag="pB")
                ssl = slice(att * 512 + (sq % 4) * 128,
                            att * 512 + (sq % 4 + 1) * 128)
                for kt in range(SKT):
                    nc.tensor.matmul(ups[:, 0:DH + 1],
                                     es_list[kt][:, ssl],
                                     V[:, kt, h, :],
                                     start=(kt == 0), stop=(kt == SKT - 1))
                nc.vector.tensor_copy(usb[:, sq, 0:DH + 1], ups[:, 0:DH + 1])

            loop_cm = tc.For_i(0, reps, 1) if reps > 1 else contextlib.nullcontext()
            with loop_cm:
                # ---------------- V sweep ----------------
                with tc.tile_pool(name="wv", bufs=1) as wvp:
                    wv = wvp.tile([128, KT, 512], BF16, tag="wv")
                    for kc4 in range(4):
                        kc = slice(kc4 * 4, (kc4 + 1) * 4)
                        nc.sync.dma_start(out=wv[:, kc], in_=wvb_t[:, kc])
                    for g in range(4):
                        xc = xsp.tile([128, KT, 512], BF16, tag="x", name=f"xv{g}")
                        for hf in range(2):
                            kc = slice(hf * 8, (hf + 1) * 8)
                            nc.sync.dma_start(
                                out=xc[:, kc],
                                in_=xTb_t[:, kc, g * 512:(g + 1) * 512])
                        for mt in range(4):
                            vp = gp.tile([128, 512], F32, tag="g")
                            for k in range(KT):
                                nc.tensor.matmul(
                                    vp[:],
                                    xc[:, k, mt * 128:(mt + 1) * 128],
                                    wv[:, k],
                                    start=(k == 0), stop=(k == KT - 1))
                            nc.vector.tensor_copy(
                                V[:, g * 4 + mt, :, 0:DH],
                                vp.rearrange("p (h d) -> p h d", h=NHEAD_G))

                with tc.tile_pool(name="wp", bufs=1) as wpp:
                    # K/Q for head 0 (prologue, unfilled)
                    cur = make_head_tiles(0)
                    for _ in gemm_head_gen(0, cur):
                        pass

                    OT0 = otp.tile([128, NHEAD_G, BLK], BF16, tag="OT0")
                    OT1 = otp.tile([128, NHEAD_G, BLK], BF16, tag="OT1")
                    wp = None

                    def proj_gen(OT, blk):
                        for nb in range(4):
                            nsl = slice(nb * 512, (nb + 1) * 512)
                            for mt in range(SQT):
                                msl = slice(blk * BLK + mt * 128,
                                            blk * BLK + (mt + 1) * 128)
                                pps = psB.tile([128, 512], F32, tag="pB")
                                for k in range(NHEAD_G):
                                    nc.tensor.matmul(
                                        pps[:],
                                        OT[:, k, mt * 128:(mt + 1) * 128],
                                        wp[:, k, nsl],
                                        start=(k == 0), stop=(k == NHEAD_G - 1))
                                ot = outsp.tile([128, 512], F32, tag="os")
                                nc.vector.tensor_copy(ot[:], pps[:])
                                nc.scalar.dma_start(out=out[msl, nsl], in_=ot[:])
                                yield

                    def stage(h, KTh, QTh, fill_early, fill_late, pending_pe):
                        es_l = {}
                        u = {}

                        def get_u(blk):
                            if blk not in u:
                                u1sb = up.tile([128, SQT, DH + 4], F32, tag="u1",
                                               name=f"u1_{h}_{blk}")
                                u2sb = up.tile([128, SQT, DH + 4], F32, tag="u2",
                                               name=f"u2_{h}_{blk}")
                                u[blk] = (u1sb, u2sb)
                            return u[blk]

                        def norm_dve(blk):
                            # DVE-only part of the normalize; the PE transposes
                            # are deferred so the in-order PE queue never waits
                            # on this chain
                            u1sb, u2sb = u[blk]
                            r1 = rp.tile([128, SQT], F32, tag="r")
                            nc.vector.reciprocal(r1[:], u1sb[:, :, DH])
                            r2n = rp.tile([128, SQT], F32, tag="r")
                            nc.vector.reciprocal(r2n[:], u2sb[:, :, DH])
                            nc.vector.tensor_scalar_mul(r2n[:], r2n[:], nlam[:])
                            ocs = []
                            for sq in range(SQT):
                                o1 = obp.tile([128, DH], F32, tag="o")
                                nc.vector.tensor_scalar_mul(
                                    o1[:], u1sb[:, sq, 0:DH], r1[:, sq:sq + 1])
                                o2 = obp.tile([128, DH], F32, tag="o")
                                nc.vector.tensor_scalar_mul(
                                    o2[:], u2sb[:, sq, 0:DH], r2n[:, sq:sq + 1])
                                oc = ocp.tile([128, DH], BF16, tag="oc")
                                nc.vector.tensor_add(oc[:], o1[:], o2[:])
                                ocs.append(oc)
                            return ocs

                        def norm_pe(blk, ocs):
                            OT = OT0 if blk == 0 else OT1
                            for sq in range(SQT):
                                tps = psB.tile([128, 256], BF16, tag="pB")
                                nc.tensor.transpose(tps[:, 0:128], ocs[sq][:],
                                                    ident[:])
                                nc.vector.tensor_copy(
                                    OT[:, h, sq * 128:(sq + 1) * 128],
                                    tps[:, 0:128])

                        def sec(s_key, a_key, f, tail_pulls=1, mid=None):
                            new = [] if s_key else None
                            if a_key:
                                a_blk, a_qc = a_key
                                u1sb, u2sb = get_u(a_blk)
                                av_list = es_l[a_key]
                            for i in range(8):
                                if s_key:
                                    s_blk, s_qc = s_key
                                    new.append(score_tile(KTh, QTh, s_blk, s_qc,
                                                          2 * i))
                                    new.append(score_tile(KTh, QTh, s_blk, s_qc,
                                                          2 * i + 1))
                                if a_key:
                                    att, sql = divmod(i, 4)
                                    sq = a_qc * 4 + sql
                                    usb = u1sb if att == 0 else u2sb
                                    av_group(av_list, h, att, sq, usb)
                                if i == 4 and mid is not None:
                                    mid()
                                _pull(f, tail_pulls)
                            if s_key:
                                es_l[s_key] = new

                        # S(0,0) with filler; previous stage's deferred
                        # transposes land behind the first few score tiles
                        es00 = []
                        for kt in range(SKT):
                            es00.append(score_tile(KTh, QTh, 0, 0, kt))
                            if kt == 2 and pending_pe is not None:
                                pending_pe()
                            _pull(fill_early, 2)
                        es_l[(0, 0)] = es00
                        sec((0, 1), (0, 0), fill_early)
                        sec((1, 0), (0, 1), fill_early)
                        ocs0 = norm_dve(0)
                        if h == NHEAD_G - 1:
                            # proj(blk0) pulls inside the next sec read OT0:
                            # its transposes must precede them in the PE queue
                            norm_pe(0, ocs0)
                            sec((1, 1), (1, 0), fill_late)
                        else:
                            sec((1, 1), (1, 0), fill_late,
                                mid=lambda: norm_pe(0, ocs0))
                        sec(None, (1, 1), fill_late, tail_pulls=3)
                        ocs1 = norm_dve(1)
                        return lambda: norm_pe(1, ocs1)

                    pending = None
                    for h in range(NHEAD_G):
                        if h < NHEAD_G - 1:
                            nxt = make_head_tiles(h + 1)
                            fe = gemm_head_gen(h + 1, nxt)
                            fl = fe
                        else:
                            wp = wpp.tile([128, NHEAD_G, DIM], BF16, tag="wp")
                            for hh in range(NHEAD_G):
                                nc.sync.dma_start(out=wp[:, hh], in_=wpb_t[:, hh])
                            fe = iter(())
                            fl = proj_gen(OT0, 0)
                        pending = stage(h, cur[0], cur[1], fe, fl, pending)
                        while _pull(fe):
                            pass
                        if h == NHEAD_G - 1:
                            while _pull(fl):
                                pass
                        else:
                            cur = nxt
                    pending()
                    for _ in proj_gen(OT1, 1):
                        pass

    nc.compile()
    return nc


_CACHE = {}


def _get_program(reps=1):
    key = f"nc{reps}"
    if key not in _CACHE:
        _CACHE[key] = build_program(reps)
    return _CACHE[key]


def shard_inputs(inputs):
    """Full-input dict -> per-core in_maps for run_bass_kernel_spmd."""
    x = np.asarray(inputs["x"], dtype=np.float32)
    w_qkv = np.asarray(inputs["w_qkv"], dtype=np.float32)
    w_proj = np.asarray(inputs["w_proj"], dtype=np.float32)
    lambda_q1 = np.asarray(inputs["lambda_q1"], dtype=np.float32)
    lambda_k1 = np.asarray(inputs["lambda_k1"], dtype=np.float32)
    lambda_q2 = np.asarray(inputs["lambda_q2"], dtype=np.float32)
    lambda_k2 = np.asarray(inputs["lambda_k2"], dtype=np.float32)
    li = np.float32(np.asarray(inputs["layer_idx"]))

    B = x.shape[0]
    H = 16

    layer_factor = np.clip(li * np.float32(0.3), np.float32(0.0), np.float32(5.0))
    lam_init = np.float32(0.8) - np.float32(0.6) * np.exp(-layer_factor)
    l1 = np.clip(np.sum(lambda_q1 * lambda_k1), -10.0, 10.0).astype(np.float32)
    l2 = np.clip(np.sum(lambda_q2 * lambda_k2), -10.0, 10.0).astype(np.float32)
    lam = np.clip(np.exp(l1) - np.exp(l2) + lam_init, 0.1, 5.0).astype(np.float32)

    xT = [np.ascontiguousarray(x[b].T) for b in range(B)]
    xTb = [t.astype(ml_dtypes.bfloat16) for t in xT]
    neg_lam = np.array([[-lam]], dtype=np.float32)

    in_maps = []
    for c in range(8):
        b = c // 4
        g = c % 4
        h0 = g * NHEAD_G
        cq = slice(h0 * DH, (h0 + NHEAD_G) * DH)
        ck = slice(H * DH + h0 * DH, H * DH + (h0 + NHEAD_G) * DH)
        cv = slice(2 * H * DH + h0 * DH, 2 * H * DH + (h0 + NHEAD_G) * DH)
        in_maps.append({
            "xTb": xTb[b],
            "wq": (np.ascontiguousarray(w_qkv[:, cq])
                   * np.float32(SCALE)).astype(ml_dtypes.bfloat16),
            "wk": np.ascontiguousarray(w_qkv[:, ck]).astype(ml_dtypes.bfloat16),
            "wvb": np.ascontiguousarray(w_qkv[:, cv]).astype(ml_dtypes.bfloat16),
            "wpb": np.ascontiguousarray(
                w_proj[h0 * DH:(h0 + NHEAD_G) * DH, :]).astype(ml_dtypes.bfloat16),
            "neg_lam": neg_lam,
        })
    return in_maps


def kernel(x, w_qkv, w_proj, b_proj, lambda_q1, lambda_k1, lambda_q2, lambda_k2,
           layer_idx):
    inputs = dict(x=x, w_qkv=w_qkv, w_proj=w_proj, b_proj=b_proj,
                  lambda_q1=lambda_q1, lambda_k1=lambda_k1,
                  lambda_q2=lambda_q2, lambda_k2=lambda_k2, layer_idx=layer_idx)
    in_maps = shard_inputs(inputs)
    b_proj = np.asarray(b_proj, dtype=np.float32)
    B = np.asarray(x).shape[0]

    nc = _get_program()
    last_err = None
    for attempt in range(3):
        try:
            res = run_bass_kernel_spmd(nc, in_maps, list(range(8)))
            break
        except Exception as e:  # noqa: BLE001
            last_err = e
    else:
        raise last_err

    out = np.empty((B, S, DIM), dtype=np.float32)
    for b in range(B):
        acc = res.results[4 * b]["out"].copy()
        for g in range(1, 4):
            acc += res.results[4 * b + g]["out"]
        out[b] = acc + b_proj
    return out
